# revision 9
# baseline (speedup 1.0000x reference)
"""FeaStConv dual-branch GNN message passing on 8 Trainium2 NeuronCores.

Sharding: branch v on cores 0-3, branch f on cores 4-7; each core owns a
12500-node destination range. Host reorders edges by destination block
(64 nodes), pre-gathers transposed source/dest features (bf16) plus a
block-local destination one-hot (fp8, exact 0/1). Device does all float
math. One fused matmul per tile computes both x@W (128 cols) and
(x_s-x_d)@U (4 cols) into bank-packed PSUM slices; scatter matmuls run one
superchunk behind so the q*xjw multiply (split between Vector and
Scalar-copy+GpSimd) has a full superchunk of slack.
"""
import sys, types
import numpy as np

sys.path.insert(0, '/opt/trn_rl_repo')

N = 50000
IN_CH = 64
HEADS = 4
OUT_CH = 32
P = 128
NPC = 12500           # nodes per core
BLK = 64              # dst nodes per block
NBLK = 196            # blocks per core (196*64 = 12544)
NPAD = NBLK * BLK
CH = 9                # tiles per chunk (3 PSUM banks, 3 slices of 132 each)
NCHK = 4              # chunks per superchunk
SCT = CH * NCHK       # tiles per superchunk (36)
HLF = SCT // 2        # softmax granularity: half superchunk (2 chunks), so
                      # only 2 pA bufs are needed per softmax (no psA cycle)
BANK = 512            # fp32 elems per PSUM bank
WUC = 132             # fused rhs cols: 128 xjw + 4 z
NCORES = 8
G_NUM = 2             # chunks with ci_g % G_DEN < G_NUM take the gpsimd path
G_DEN = 5


def _register_ntff_hook():
    import antenv
    if "antenv.axon_hooks" in sys.modules:
        return
    mod = types.ModuleType("antenv.axon_hooks")
    _h = [None]
    mod.set_axon_ntff_profile_hook = lambda h: _h.__setitem__(0, h)
    mod.get_axon_ntff_profile_hook = lambda: _h[0]
    sys.modules["antenv.axon_hooks"] = mod
    antenv.axon_hooks = mod
    if "/root/.axon_site" not in sys.path:
        sys.path.insert(0, "/root/.axon_site")
    try:
        from trn_agent_boot.trn_boot import _ntff_profile_via_ctypes
        mod.set_axon_ntff_profile_hook(_ntff_profile_via_ctypes('/opt/axon/libaxon_pjrt.so'))
    except Exception:
        pass


def _prep_core(x16, src, dst, lo):
    sel = (dst >= lo) & (dst < lo + NPC)
    s = src[sel]
    d = (dst[sel] - lo).astype(np.int64)
    order = np.argsort(d, kind='stable')
    s = s[order]
    d = d[order]
    blk = d >> 6
    cnt = np.bincount(blk, minlength=NBLK).astype(np.int64)
    deg = np.bincount(d, minlength=NPAD).astype(np.float32)
    return {"s": s, "d": d, "cnt": cnt, "deg": deg}


def _build_core_arrays(ml, core, TPB, base, NT):
    import ml_dtypes
    x16, W, U, c, b = core["x16"], core["W"], core["U"], core["c"], core["b"]
    s, d, cnt = core["g"]["s"], core["g"]["d"], core["g"]["cnt"]
    E_pad = NT * P
    srcg = np.zeros(E_pad, np.int64)
    dstg = np.zeros(E_pad, np.int64)
    dl = np.full(E_pad, -1.0, np.float32)
    cstart = np.concatenate([[0], np.cumsum(cnt)])
    for k in range(NBLK):
        n_k = int(cnt[k])
        if n_k == 0:
            continue
        p0 = base[k] * P
        srcg[p0:p0 + n_k] = s[cstart[k]:cstart[k] + n_k]
        dstg[p0:p0 + n_k] = d[cstart[k]:cstart[k] + n_k] + core["lo"]
        dl[p0:p0 + n_k] = (d[cstart[k]:cstart[k] + n_k] - BLK * k).astype(np.float32)
    xsd = np.empty((P, E_pad), ml_dtypes.bfloat16)
    xsd[:IN_CH, :] = x16[srcg].T
    xsd[IN_CH:, :] = x16[dstg].T
    dlr = dl.reshape(NT, P)
    oh = (dlr[:, :, None] == np.arange(BLK, dtype=np.float32)[None, None, :])
    ohm = np.ascontiguousarray(
        oh.transpose(1, 0, 2).reshape(P, NT * BLK)).astype(ml_dtypes.float8_e4m3fn)
    # fused rhs: cols 0:128 = W (top rows), cols 128:132 = [U; -U]
    WU = np.zeros((P, WUC), np.float32)
    WU[:IN_CH, :P] = W
    WU[:IN_CH, P:] = U
    WU[IN_CH:, P:] = -U
    degp = np.ascontiguousarray(core["g"]["deg"].reshape(NBLK // 2, P).T)  # [128, 98]
    return {
        "xsd": xsd,
        "ohm": ohm,
        "wcm": WU.astype(ml_dtypes.bfloat16),
        "crep": np.tile(c[None, :], (P, 1)).astype(np.float32),
        "brep": np.tile(b[None, :], (P, 1)).astype(np.float32),
        "degp": degp.astype(np.float32),
    }


def _build_program(TPB, NT):
    import concourse.bass as bass
    import concourse.mybir as mybir
    import concourse.bacc as bacc
    from concourse.tile import TileContext

    dt = mybir.dt
    NSC = NT // SCT
    blk_of = np.repeat(np.arange(NBLK), TPB)
    t0 = np.concatenate([[0], np.cumsum(TPB)])

    nc = bacc.Bacc("TRN2", target_bir_lowering=False, debug=False, num_devices=NCORES)
    xsd_d = nc.dram_tensor("xsd", [P, NT * P], dt.bfloat16, kind="ExternalInput").ap()
    ohm_d = nc.dram_tensor("ohm", [P, NT * BLK], dt.float8e4, kind="ExternalInput").ap()
    wcm_d = nc.dram_tensor("wcm", [P, WUC], dt.bfloat16, kind="ExternalInput").ap()
    crep_d = nc.dram_tensor("crep", [P, 4], dt.float32, kind="ExternalInput").ap()
    brep_d = nc.dram_tensor("brep", [P, OUT_CH], dt.float32, kind="ExternalInput").ap()
    degp_d = nc.dram_tensor("degp", [P, NBLK // 2], dt.float32, kind="ExternalInput").ap()
    out_d = nc.dram_tensor("out", [NPAD, OUT_CH], dt.float32, kind="ExternalOutput").ap()

    def APn(t, dims, off=0):
        a = t[:]
        return bass.AP(a.tensor, a.offset + off, [a.ap[0]] + dims)

    # PSUM slice offset for tile i within a chunk: 3 slices of 132 per bank
    def soff(i):
        return (i // 3) * BANK + (i % 3) * WUC

    with TileContext(nc) as tc:
        with tc.tile_pool(name="const", bufs=1) as cp, \
             tc.tile_pool(name="mega", bufs=3) as mp, \
             tc.tile_pool(name="ohp", bufs=3) as op_, \
             tc.tile_pool(name="work", bufs=10) as wp, \
             tc.tile_pool(name="cpb", bufs=3) as cb, \
             tc.tile_pool(name="qp", bufs=2) as qp, \
             tc.tile_pool(name="fin", bufs=2) as fp, \
             tc.tile_pool(name="finacc", bufs=1) as fap, \
             tc.tile_pool(name="psA", bufs=2, space="PSUM") as psA, \
             tc.tile_pool(name="psG", bufs=2, space="PSUM") as psG:

            wcm = cp.tile([P, WUC], dt.bfloat16)
            crep = cp.tile([P, 4], dt.float32)
            brep = cp.tile([P, OUT_CH], dt.float32)
            degp = cp.tile([P, NBLK // 2], dt.float32)
            expc = cp.tile([P, 4], dt.float32)
            nc.sync.dma_start(out=wcm[:], in_=wcm_d[:])
            nc.sync.dma_start(out=crep[:], in_=crep_d[:])
            nc.sync.dma_start(out=brep[:], in_=brep_d[:])
            nc.sync.dma_start(out=degp[:], in_=degp_d[:])
            nc.scalar.activation(expc[:], crep[:], mybir.ActivationFunctionType.Exp)

            fin = fap.tile([P, (NBLK // 2) * P], dt.float32)

            NH = NBLK // 2
            FIN_BOUNDS = [25, 50, 75, NH]

            def emit_finale(g0, g1):
                ng = g1 - g0
                hs = fp.tile([P, ng * OUT_CH], dt.float32, tag="hs", name="hs")
                h2 = fp.tile([P, ng * OUT_CH], dt.float32, tag="h2", name="h2")
                nc.vector.tensor_tensor(
                    out=APn(hs, [[32, ng], [1, 32]]),
                    in0=APn(fin, [[P, ng], [1, 32]], off=g0 * P),
                    in1=APn(fin, [[P, ng], [1, 32]], off=g0 * P + 32),
                    op=mybir.AluOpType.add)
                nc.vector.tensor_tensor(
                    out=APn(h2, [[32, ng], [1, 32]]),
                    in0=APn(fin, [[P, ng], [1, 32]], off=g0 * P + 64),
                    in1=APn(fin, [[P, ng], [1, 32]], off=g0 * P + 96),
                    op=mybir.AluOpType.add)
                nc.vector.tensor_tensor(
                    out=APn(hs, [[32, ng], [1, 32]]),
                    in0=APn(hs, [[32, ng], [1, 32]]),
                    in1=APn(h2, [[32, ng], [1, 32]]),
                    op=mybir.AluOpType.add)
                dmx = fp.tile([P, ng], dt.float32, tag="dmx", name="dmx")
                nc.vector.tensor_scalar(out=dmx[:], in0=degp[:, g0:g1],
                                        scalar1=1.0, scalar2=None,
                                        op0=mybir.AluOpType.max)
                drc = fp.tile([P, ng], dt.float32, tag="drc", name="drc")
                nc.vector.reciprocal(out=drc[:], in_=dmx[:])
                o1 = fp.tile([P, ng * OUT_CH], dt.float32, tag="o1", name="o1")
                nc.vector.tensor_tensor(
                    out=APn(o1, [[32, ng], [1, 32]]),
                    in0=APn(hs, [[32, ng], [1, 32]]),
                    in1=APn(drc, [[1, ng], [0, 32]]),
                    op=mybir.AluOpType.mult)
                nc.vector.tensor_tensor(
                    out=APn(o1, [[32, ng], [1, 32]]),
                    in0=APn(o1, [[32, ng], [1, 32]]),
                    in1=APn(brep, [[0, ng], [1, 32]]),
                    op=mybir.AluOpType.add)
                o2 = fp.tile([P, ng * OUT_CH], dt.float32, tag="o2", name="o2")
                nc.vector.tensor_scalar(out=o2[:], in0=o1[:], scalar1=0.2,
                                        scalar2=None, op0=mybir.AluOpType.mult)
                nc.vector.tensor_tensor(out=o1[:], in0=o1[:], in1=o2[:],
                                        op=mybir.AluOpType.max)
                out_ap = bass.AP(out_d.tensor, out_d.offset + g0 * P * OUT_CH,
                                 [[OUT_CH, P], [P * OUT_CH, ng], [1, OUT_CH]])
                nc.sync.dma_start(out=out_ap, in_=APn(o1, [[32, ng], [1, 32]]))

            state = {"acc": None}

            def emit_scatter(sc, stgs, ohm):
                for ci in range(NCHK):
                    stg = stgs[ci]
                    for i in range(CH):
                        t = sc * SCT + ci * CH + i
                        k = int(blk_of[t])
                        if k % 2 == 0 and t == t0[k]:
                            state["acc"] = psG.tile([P, P], dt.float32, tag="acc", name="acc")
                        acc = state["acc"]
                        half = (k % 2) * BLK
                        nc.tensor.matmul(
                            out=acc[half:half + BLK, :],
                            lhsT=ohm[:, (ci * CH + i) * BLK:(ci * CH + i + 1) * BLK],
                            rhs=stg[:, i * P:(i + 1) * P],
                            start=(t == t0[k]), stop=(t == t0[k + 1] - 1))
                        if k % 2 == 1 and t == t0[k + 1] - 1:
                            m = k // 2
                            nc.scalar.copy(out=fin[:, m * P:(m + 1) * P], in_=acc[:])
                            if (m + 1) in FIN_BOUNDS:
                                emit_finale(FIN_BOUNDS[FIN_BOUNDS.index(m + 1) - 1]
                                            if FIN_BOUNDS.index(m + 1) > 0 else 0,
                                            m + 1)

            prev = None
            for sc in range(NSC):
                xm = mp.tile([P, SCT * P], dt.bfloat16, tag="xm", name="xm")
                nc.sync.dma_start(out=xm[:], in_=xsd_d[:, sc * SCT * P:(sc + 1) * SCT * P])
                ohm = op_.tile([P, SCT * BLK], dt.float8e4, tag="ohm", name="ohm")
                nc.sync.dma_start(out=ohm[:], in_=ohm_d[:, sc * SCT * BLK:(sc + 1) * SCT * BLK])
                qe = qp.tile([P, SCT * 4], dt.float32, tag="qe", name="qe")
                qb = qp.tile([P, SCT * 4], dt.float32, tag="qb", name="qb")
                qd = qp.tile([P, SCT * 2], dt.float32, tag="qd", name="qd")
                den = qp.tile([P, SCT], dt.float32, tag="den", name="den")
                rec = qp.tile([P, SCT], dt.float32, tag="rec", name="rec")

                cur = []
                for half in range(2):
                    chunk_data = []
                    for ci in (2 * half, 2 * half + 1):
                        pA = psA.tile([P, 3 * BANK], dt.float32, tag="pA", name="pA")
                        for i in range(CH):
                            lhs = xm[:, (ci * CH + i) * P:(ci * CH + i + 1) * P]
                            nc.tensor.matmul(out=pA[:, soff(i):soff(i) + WUC], lhsT=lhs,
                                             rhs=wcm[:], start=True, stop=True)
                        # exp of the 4 z-cols of each of the 9 slices -> qe cols
                        nc.scalar.activation(
                            out=qe[:, ci * CH * 4:(ci + 1) * CH * 4],
                            in_=APn(pA, [[BANK, 3], [WUC, 3], [1, 4]], off=P),
                            func=mybir.ActivationFunctionType.Exp)
                        chunk_data.append(pA)

                    # softmax over this half superchunk
                    ho4 = half * HLF * 4
                    nc.vector.tensor_tensor(
                        out=APn(qb, [[4, HLF], [1, 4]], off=ho4),
                        in0=APn(qe, [[4, HLF], [1, 4]], off=ho4),
                        in1=APn(expc, [[0, HLF], [1, 4]]),
                        op=mybir.AluOpType.mult)
                    nc.vector.tensor_tensor(
                        out=APn(qd, [[2, HLF], [1, 2]], off=half * HLF * 2),
                        in0=APn(qb, [[4, HLF], [1, 2]], off=ho4),
                        in1=APn(qb, [[4, HLF], [1, 2]], off=ho4 + 2),
                        op=mybir.AluOpType.add)
                    nc.vector.tensor_tensor(
                        out=APn(den, [[1, HLF], [1, 1]], off=half * HLF),
                        in0=APn(qd, [[2, HLF], [1, 1]], off=half * HLF * 2),
                        in1=APn(qd, [[2, HLF], [1, 1]], off=half * HLF * 2 + 1),
                        op=mybir.AluOpType.add)
                    nc.vector.reciprocal(out=rec[:, half * HLF:(half + 1) * HLF],
                                         in_=den[:, half * HLF:(half + 1) * HLF])
                    nc.vector.tensor_tensor(
                        out=APn(qe, [[4, HLF], [1, 4]], off=ho4),
                        in0=APn(qb, [[4, HLF], [1, 4]], off=ho4),
                        in1=APn(rec, [[1, HLF], [0, 4]], off=half * HLF),
                        op=mybir.AluOpType.mult)

                    for ci in (2 * half, 2 * half + 1):
                        pA = chunk_data[ci - 2 * half]
                        ci_g = sc * NCHK + ci
                        stg = wp.tile([P, CH * P], dt.bfloat16, tag="stg", name="stg")
                        if ci_g % G_DEN < G_NUM:
                            sA = cb.tile([P, 3 * BANK], dt.bfloat16, tag="sA", name="sA")
                            nc.scalar.copy(out=sA[:], in_=pA[:])
                            for g in range(3):
                                nc.gpsimd.tensor_tensor(
                                    out=APn(stg, [[P, 3], [32, 4], [1, 32]], off=g * 3 * P),
                                    in0=APn(sA, [[WUC, 3], [32, 4], [1, 32]], off=g * BANK),
                                    in1=APn(qe, [[4, 3], [1, 4], [0, 32]],
                                            off=(ci * CH + g * 3) * 4),
                                    op=mybir.AluOpType.mult)
                        else:
                            for g in range(3):
                                nc.vector.tensor_tensor(
                                    out=APn(stg, [[P, 3], [32, 4], [1, 32]], off=g * 3 * P),
                                    in0=APn(pA, [[WUC, 3], [32, 4], [1, 32]], off=g * BANK),
                                    in1=APn(qe, [[4, 3], [1, 4], [0, 32]],
                                            off=(ci * CH + g * 3) * 4),
                                    op=mybir.AluOpType.mult)
                        cur.append(stg)

                if prev is not None:
                    emit_scatter(sc - 1, prev[0], prev[1])
                prev = (cur, ohm)
            emit_scatter(NSC - 1, prev[0], prev[1])
    nc.compile()
    return nc


def kernel(x_v, edge_index_v, x_f, edge_index_f, Wv, Uv, cv, bv, Wf, Uf, cf, bf):
    _register_ntff_hook()
    import ml_dtypes
    from concourse import bass_utils

    x_v = np.asarray(x_v, np.float32)
    x_f = np.asarray(x_f, np.float32)
    cores = []
    for bi, (x, ei, W, U, c, b) in enumerate([
            (x_v, edge_index_v, Wv, Uv, cv, bv),
            (x_f, edge_index_f, Wf, Uf, cf, bf)]):
        ei = np.asarray(ei)
        s0, d0 = ei[0].astype(np.int64), ei[1].astype(np.int64)
        m = s0 != d0
        loops = np.arange(N, dtype=np.int64)
        src = np.concatenate([s0[m], loops])
        dst = np.concatenate([d0[m], loops])
        x16 = x.astype(ml_dtypes.bfloat16)
        for j in range(4):
            lo = j * NPC
            cores.append({
                "x16": x16, "W": np.asarray(W, np.float32),
                "U": np.asarray(U, np.float32), "c": np.asarray(c, np.float32),
                "b": np.asarray(b, np.float32), "lo": lo,
                "g": _prep_core(x16, src, dst, lo),
            })

    tn = np.stack([np.ceil(c["g"]["cnt"] / P).astype(np.int64) for c in cores])
    TPB = tn.max(axis=0)
    TPB = np.maximum(TPB, 1)
    NT = int(TPB.sum())
    pad = (-NT) % SCT
    TPB[NBLK - 1] += pad
    NT += pad
    base = np.concatenate([[0], np.cumsum(TPB)])[:-1]

    in_maps = []
    for c in cores:
        arrs = _build_core_arrays(None, c, TPB, base, NT)
        in_maps.append(arrs)

    nc = _build_program(TPB, NT)
    res = bass_utils.run_bass_kernel_spmd(
        nc, in_maps, core_ids=list(range(NCORES)),
        trace=bool(int(__import__("os").environ.get("KERNEL_TRACE", "0"))))
    kernel.last_result = res
    out_v = np.concatenate([res.results[j]["out"][:NPC] for j in range(4)])
    out_f = np.concatenate([res.results[4 + j]["out"][:NPC] for j in range(4)])
    return out_v, out_f


# revision 15
# speedup vs baseline: 1.1349x; 1.1349x over previous
"""FeaStConv dual-branch GNN message passing on 8 Trainium2 NeuronCores.

Sharding: branch v on cores 0-3, branch f on cores 4-7; each core owns a
12500-node destination range. Host reorders edges by destination block
(64 nodes), pre-gathers transposed source/dest features (bf16) plus a
block-local destination one-hot (fp8, exact 0/1). Device does all float
math. One fused matmul per tile computes both x@W (128 cols) and
(x_s-x_d)@U (4 cols) into bank-packed PSUM slices; scatter matmuls run one
superchunk behind so the q*xjw multiply (split between Vector and
Scalar-copy+GpSimd) has a full superchunk of slack.
"""
import sys, types
import numpy as np

sys.path.insert(0, '/opt/trn_rl_repo')

N = 50000
IN_CH = 64
HEADS = 4
OUT_CH = 32
P = 128
NPC = 12500           # nodes per core
BLK = 64              # dst nodes per block
NBLK = 196            # blocks per core (196*64 = 12544)
NPAD = NBLK * BLK
CH = 6                # tiles per chunk (2 PSUM banks, 3 slices of 132 each)
NCHK = 4              # chunks per superchunk
SCT = CH * NCHK       # tiles per superchunk (24)
HLF = SCT // 2        # softmax granularity: half superchunk (2 chunks), so
                      # only 2 of the 3 pA bufs are pinned per softmax -- the
                      # spare buffer lets the next half's matmuls run ahead
BANK = 512            # fp32 elems per PSUM bank
WUC = 132             # fused rhs cols: 128 xjw + 4 z
NCORES = 8
G_NUM = 2             # chunks with ci_g % G_DEN < G_NUM take the gpsimd path
G_DEN = 5


def _register_ntff_hook():
    import antenv
    if "antenv.axon_hooks" in sys.modules:
        return
    mod = types.ModuleType("antenv.axon_hooks")
    _h = [None]
    mod.set_axon_ntff_profile_hook = lambda h: _h.__setitem__(0, h)
    mod.get_axon_ntff_profile_hook = lambda: _h[0]
    sys.modules["antenv.axon_hooks"] = mod
    antenv.axon_hooks = mod
    if "/root/.axon_site" not in sys.path:
        sys.path.insert(0, "/root/.axon_site")
    try:
        from trn_agent_boot.trn_boot import _ntff_profile_via_ctypes
        mod.set_axon_ntff_profile_hook(_ntff_profile_via_ctypes('/opt/axon/libaxon_pjrt.so'))
    except Exception:
        pass


def _prep_core(x16, src, dst, lo):
    sel = (dst >= lo) & (dst < lo + NPC)
    s = src[sel]
    d = (dst[sel] - lo).astype(np.int64)
    order = np.argsort(d, kind='stable')
    s = s[order]
    d = d[order]
    blk = d >> 6
    cnt = np.bincount(blk, minlength=NBLK).astype(np.int64)
    deg = np.bincount(d, minlength=NPAD).astype(np.float32)
    return {"s": s, "d": d, "cnt": cnt, "deg": deg}


def _build_core_arrays(ml, core, TPB, base, NT):
    import ml_dtypes
    x16, W, U, c, b = core["x16"], core["W"], core["U"], core["c"], core["b"]
    s, d, cnt = core["g"]["s"], core["g"]["d"], core["g"]["cnt"]
    E_pad = NT * P
    srcg = np.zeros(E_pad, np.int64)
    dstg = np.zeros(E_pad, np.int64)
    dl = np.full(E_pad, -1.0, np.float32)
    cstart = np.concatenate([[0], np.cumsum(cnt)])
    for k in range(NBLK):
        n_k = int(cnt[k])
        if n_k == 0:
            continue
        p0 = base[k] * P
        srcg[p0:p0 + n_k] = s[cstart[k]:cstart[k] + n_k]
        dstg[p0:p0 + n_k] = d[cstart[k]:cstart[k] + n_k] + core["lo"]
        dl[p0:p0 + n_k] = (d[cstart[k]:cstart[k] + n_k] - BLK * k).astype(np.float32)
    xsd = np.empty((P, E_pad), ml_dtypes.bfloat16)
    xsd[:IN_CH, :] = x16[srcg].T
    xsd[IN_CH:, :] = x16[dstg].T
    dlr = dl.reshape(NT, P)
    oh = (dlr[:, :, None] == np.arange(BLK, dtype=np.float32)[None, None, :])
    ohm = np.ascontiguousarray(
        oh.transpose(1, 0, 2).reshape(P, NT * BLK)).astype(ml_dtypes.float8_e4m3fn)
    # fused rhs: cols 0:128 = W (top rows), cols 128:132 = [U; -U]
    WU = np.zeros((P, WUC), np.float32)
    WU[:IN_CH, :P] = W
    WU[:IN_CH, P:] = U
    WU[IN_CH:, P:] = -U
    degp = np.ascontiguousarray(core["g"]["deg"].reshape(NBLK // 2, P).T)  # [128, 98]
    return {
        "xsd": xsd,
        "ohm": ohm,
        "wcm": WU.astype(ml_dtypes.bfloat16),
        "crep": np.tile(c[None, :], (P, 1)).astype(np.float32),
        "brep": np.tile(b[None, :], (P, 1)).astype(np.float32),
        "degp": degp.astype(np.float32),
    }


def _build_program(TPB, NT):
    import concourse.bass as bass
    import concourse.mybir as mybir
    import concourse.bacc as bacc
    from concourse.tile import TileContext

    dt = mybir.dt
    NSC = NT // SCT
    blk_of = np.repeat(np.arange(NBLK), TPB)
    t0 = np.concatenate([[0], np.cumsum(TPB)])

    nc = bacc.Bacc("TRN2", target_bir_lowering=False, debug=False, num_devices=NCORES)
    xsd_d = nc.dram_tensor("xsd", [P, NT * P], dt.bfloat16, kind="ExternalInput").ap()
    ohm_d = nc.dram_tensor("ohm", [P, NT * BLK], dt.float8e4, kind="ExternalInput").ap()
    wcm_d = nc.dram_tensor("wcm", [P, WUC], dt.bfloat16, kind="ExternalInput").ap()
    crep_d = nc.dram_tensor("crep", [P, 4], dt.float32, kind="ExternalInput").ap()
    brep_d = nc.dram_tensor("brep", [P, OUT_CH], dt.float32, kind="ExternalInput").ap()
    degp_d = nc.dram_tensor("degp", [P, NBLK // 2], dt.float32, kind="ExternalInput").ap()
    out_d = nc.dram_tensor("out", [NPAD, OUT_CH], dt.float32, kind="ExternalOutput").ap()

    def APn(t, dims, off=0):
        a = t[:]
        return bass.AP(a.tensor, a.offset + off, [a.ap[0]] + dims)

    # PSUM slice offset for tile i within a chunk: 3 slices of 132 per bank
    def soff(i):
        return (i // 3) * BANK + (i % 3) * WUC

    with TileContext(nc) as tc:
        with tc.tile_pool(name="const", bufs=1) as cp, \
             tc.tile_pool(name="mega", bufs=3) as mp, \
             tc.tile_pool(name="ohp", bufs=3) as op_, \
             tc.tile_pool(name="work", bufs=10) as wp, \
             tc.tile_pool(name="cpb", bufs=3) as cb, \
             tc.tile_pool(name="qp", bufs=2) as qp, \
             tc.tile_pool(name="fin", bufs=2) as fp, \
             tc.tile_pool(name="finacc", bufs=1) as fap, \
             tc.tile_pool(name="psA", bufs=3, space="PSUM") as psA, \
             tc.tile_pool(name="psG", bufs=2, space="PSUM") as psG:

            wcm = cp.tile([P, WUC], dt.bfloat16)
            crep = cp.tile([P, 4], dt.float32)
            brep = cp.tile([P, OUT_CH], dt.float32)
            degp = cp.tile([P, NBLK // 2], dt.float32)
            expc = cp.tile([P, 4], dt.float32)
            nc.sync.dma_start(out=wcm[:], in_=wcm_d[:])
            nc.sync.dma_start(out=crep[:], in_=crep_d[:])
            nc.sync.dma_start(out=brep[:], in_=brep_d[:])
            nc.sync.dma_start(out=degp[:], in_=degp_d[:])
            nc.scalar.activation(expc[:], crep[:], mybir.ActivationFunctionType.Exp)

            fin = fap.tile([P, (NBLK // 2) * P], dt.float32)

            NH = NBLK // 2
            FIN_BOUNDS = [25, 50, 75, NH]

            def emit_finale(g0, g1):
                ng = g1 - g0
                hs = fp.tile([P, ng * OUT_CH], dt.float32, tag="hs", name="hs")
                h2 = fp.tile([P, ng * OUT_CH], dt.float32, tag="h2", name="h2")
                nc.vector.tensor_tensor(
                    out=APn(hs, [[32, ng], [1, 32]]),
                    in0=APn(fin, [[P, ng], [1, 32]], off=g0 * P),
                    in1=APn(fin, [[P, ng], [1, 32]], off=g0 * P + 32),
                    op=mybir.AluOpType.add)
                nc.vector.tensor_tensor(
                    out=APn(h2, [[32, ng], [1, 32]]),
                    in0=APn(fin, [[P, ng], [1, 32]], off=g0 * P + 64),
                    in1=APn(fin, [[P, ng], [1, 32]], off=g0 * P + 96),
                    op=mybir.AluOpType.add)
                nc.vector.tensor_tensor(
                    out=APn(hs, [[32, ng], [1, 32]]),
                    in0=APn(hs, [[32, ng], [1, 32]]),
                    in1=APn(h2, [[32, ng], [1, 32]]),
                    op=mybir.AluOpType.add)
                dmx = fp.tile([P, ng], dt.float32, tag="dmx", name="dmx")
                nc.vector.tensor_scalar(out=dmx[:], in0=degp[:, g0:g1],
                                        scalar1=1.0, scalar2=None,
                                        op0=mybir.AluOpType.max)
                drc = fp.tile([P, ng], dt.float32, tag="drc", name="drc")
                nc.vector.reciprocal(out=drc[:], in_=dmx[:])
                o1 = fp.tile([P, ng * OUT_CH], dt.float32, tag="o1", name="o1")
                nc.vector.tensor_tensor(
                    out=APn(o1, [[32, ng], [1, 32]]),
                    in0=APn(hs, [[32, ng], [1, 32]]),
                    in1=APn(drc, [[1, ng], [0, 32]]),
                    op=mybir.AluOpType.mult)
                nc.vector.tensor_tensor(
                    out=APn(o1, [[32, ng], [1, 32]]),
                    in0=APn(o1, [[32, ng], [1, 32]]),
                    in1=APn(brep, [[0, ng], [1, 32]]),
                    op=mybir.AluOpType.add)
                o2 = fp.tile([P, ng * OUT_CH], dt.float32, tag="o2", name="o2")
                nc.vector.tensor_scalar(out=o2[:], in0=o1[:], scalar1=0.2,
                                        scalar2=None, op0=mybir.AluOpType.mult)
                nc.vector.tensor_tensor(out=o1[:], in0=o1[:], in1=o2[:],
                                        op=mybir.AluOpType.max)
                out_ap = bass.AP(out_d.tensor, out_d.offset + g0 * P * OUT_CH,
                                 [[OUT_CH, P], [P * OUT_CH, ng], [1, OUT_CH]])
                nc.sync.dma_start(out=out_ap, in_=APn(o1, [[32, ng], [1, 32]]))

            state = {"acc": None}

            def emit_scatter(sc, stgs, ohm):
                for ci in range(NCHK):
                    stg = stgs[ci]
                    for i in range(CH):
                        t = sc * SCT + ci * CH + i
                        k = int(blk_of[t])
                        if k % 2 == 0 and t == t0[k]:
                            state["acc"] = psG.tile([P, P], dt.float32, tag="acc", name="acc")
                        acc = state["acc"]
                        half = (k % 2) * BLK
                        nc.tensor.matmul(
                            out=acc[half:half + BLK, :],
                            lhsT=ohm[:, (ci * CH + i) * BLK:(ci * CH + i + 1) * BLK],
                            rhs=stg[:, i * P:(i + 1) * P],
                            start=(t == t0[k]), stop=(t == t0[k + 1] - 1))
                        if k % 2 == 1 and t == t0[k + 1] - 1:
                            m = k // 2
                            nc.scalar.copy(out=fin[:, m * P:(m + 1) * P], in_=acc[:])
                            if (m + 1) in FIN_BOUNDS:
                                emit_finale(FIN_BOUNDS[FIN_BOUNDS.index(m + 1) - 1]
                                            if FIN_BOUNDS.index(m + 1) > 0 else 0,
                                            m + 1)

            prev = None
            for sc in range(NSC):
                xm = mp.tile([P, SCT * P], dt.bfloat16, tag="xm", name="xm")
                nc.sync.dma_start(out=xm[:], in_=xsd_d[:, sc * SCT * P:(sc + 1) * SCT * P])
                ohm = op_.tile([P, SCT * BLK], dt.float8e4, tag="ohm", name="ohm")
                nc.sync.dma_start(out=ohm[:], in_=ohm_d[:, sc * SCT * BLK:(sc + 1) * SCT * BLK])
                qe = qp.tile([P, SCT * 4], dt.float32, tag="qe", name="qe")
                qb = qp.tile([P, SCT * 4], dt.float32, tag="qb", name="qb")
                qd = qp.tile([P, SCT * 2], dt.float32, tag="qd", name="qd")
                den = qp.tile([P, SCT], dt.float32, tag="den", name="den")
                rec = qp.tile([P, SCT], dt.float32, tag="rec", name="rec")

                cur = []
                for half in range(2):
                    chunk_data = []
                    for ci in (2 * half, 2 * half + 1):
                        pA = psA.tile([P, 2 * BANK], dt.float32, tag="pA", name="pA")
                        for i in range(CH):
                            lhs = xm[:, (ci * CH + i) * P:(ci * CH + i + 1) * P]
                            nc.tensor.matmul(out=pA[:, soff(i):soff(i) + WUC], lhsT=lhs,
                                             rhs=wcm[:], start=True, stop=True)
                        # exp of the 4 z-cols of each of the 6 slices -> qe cols
                        nc.scalar.activation(
                            out=qe[:, ci * CH * 4:(ci + 1) * CH * 4],
                            in_=APn(pA, [[BANK, 2], [WUC, 3], [1, 4]], off=P),
                            func=mybir.ActivationFunctionType.Exp)
                        chunk_data.append(pA)

                    # softmax over this half superchunk
                    ho4 = half * HLF * 4
                    nc.vector.tensor_tensor(
                        out=APn(qb, [[4, HLF], [1, 4]], off=ho4),
                        in0=APn(qe, [[4, HLF], [1, 4]], off=ho4),
                        in1=APn(expc, [[0, HLF], [1, 4]]),
                        op=mybir.AluOpType.mult)
                    nc.vector.tensor_tensor(
                        out=APn(qd, [[2, HLF], [1, 2]], off=half * HLF * 2),
                        in0=APn(qb, [[4, HLF], [1, 2]], off=ho4),
                        in1=APn(qb, [[4, HLF], [1, 2]], off=ho4 + 2),
                        op=mybir.AluOpType.add)
                    nc.vector.tensor_tensor(
                        out=APn(den, [[1, HLF], [1, 1]], off=half * HLF),
                        in0=APn(qd, [[2, HLF], [1, 1]], off=half * HLF * 2),
                        in1=APn(qd, [[2, HLF], [1, 1]], off=half * HLF * 2 + 1),
                        op=mybir.AluOpType.add)
                    nc.vector.reciprocal(out=rec[:, half * HLF:(half + 1) * HLF],
                                         in_=den[:, half * HLF:(half + 1) * HLF])
                    nc.vector.tensor_tensor(
                        out=APn(qe, [[4, HLF], [1, 4]], off=ho4),
                        in0=APn(qb, [[4, HLF], [1, 4]], off=ho4),
                        in1=APn(rec, [[1, HLF], [0, 4]], off=half * HLF),
                        op=mybir.AluOpType.mult)

                    for ci in (2 * half, 2 * half + 1):
                        pA = chunk_data[ci - 2 * half]
                        ci_g = sc * NCHK + ci
                        stg = wp.tile([P, CH * P], dt.bfloat16, tag="stg", name="stg")
                        if ci_g % G_DEN < G_NUM:
                            sA = cb.tile([P, 2 * BANK], dt.bfloat16, tag="sA", name="sA")
                            nc.scalar.copy(out=sA[:], in_=pA[:])
                            for g in range(2):
                                nc.gpsimd.tensor_tensor(
                                    out=APn(stg, [[P, 3], [32, 4], [1, 32]], off=g * 3 * P),
                                    in0=APn(sA, [[WUC, 3], [32, 4], [1, 32]], off=g * BANK),
                                    in1=APn(qe, [[4, 3], [1, 4], [0, 32]],
                                            off=(ci * CH + g * 3) * 4),
                                    op=mybir.AluOpType.mult)
                        else:
                            for g in range(2):
                                nc.vector.tensor_tensor(
                                    out=APn(stg, [[P, 3], [32, 4], [1, 32]], off=g * 3 * P),
                                    in0=APn(pA, [[WUC, 3], [32, 4], [1, 32]], off=g * BANK),
                                    in1=APn(qe, [[4, 3], [1, 4], [0, 32]],
                                            off=(ci * CH + g * 3) * 4),
                                    op=mybir.AluOpType.mult)
                        cur.append(stg)

                if prev is not None:
                    emit_scatter(sc - 1, prev[0], prev[1])
                prev = (cur, ohm)
            emit_scatter(NSC - 1, prev[0], prev[1])
    nc.compile()
    return nc


def kernel(x_v, edge_index_v, x_f, edge_index_f, Wv, Uv, cv, bv, Wf, Uf, cf, bf):
    _register_ntff_hook()
    import ml_dtypes
    from concourse import bass_utils

    x_v = np.asarray(x_v, np.float32)
    x_f = np.asarray(x_f, np.float32)
    cores = []
    for bi, (x, ei, W, U, c, b) in enumerate([
            (x_v, edge_index_v, Wv, Uv, cv, bv),
            (x_f, edge_index_f, Wf, Uf, cf, bf)]):
        ei = np.asarray(ei)
        s0, d0 = ei[0].astype(np.int64), ei[1].astype(np.int64)
        m = s0 != d0
        loops = np.arange(N, dtype=np.int64)
        src = np.concatenate([s0[m], loops])
        dst = np.concatenate([d0[m], loops])
        x16 = x.astype(ml_dtypes.bfloat16)
        for j in range(4):
            lo = j * NPC
            cores.append({
                "x16": x16, "W": np.asarray(W, np.float32),
                "U": np.asarray(U, np.float32), "c": np.asarray(c, np.float32),
                "b": np.asarray(b, np.float32), "lo": lo,
                "g": _prep_core(x16, src, dst, lo),
            })

    tn = np.stack([np.ceil(c["g"]["cnt"] / P).astype(np.int64) for c in cores])
    TPB = tn.max(axis=0)
    TPB = np.maximum(TPB, 1)
    NT = int(TPB.sum())
    pad = (-NT) % SCT
    TPB[NBLK - 1] += pad
    NT += pad
    base = np.concatenate([[0], np.cumsum(TPB)])[:-1]

    in_maps = []
    for c in cores:
        arrs = _build_core_arrays(None, c, TPB, base, NT)
        in_maps.append(arrs)

    nc = _build_program(TPB, NT)
    res = bass_utils.run_bass_kernel_spmd(
        nc, in_maps, core_ids=list(range(NCORES)),
        trace=bool(int(__import__("os").environ.get("KERNEL_TRACE", "0"))))
    kernel.last_result = res
    out_v = np.concatenate([res.results[j]["out"][:NPC] for j in range(4)])
    out_f = np.concatenate([res.results[4 + j]["out"][:NPC] for j in range(4)])
    return out_v, out_f


# revision 19
# speedup vs baseline: 1.7025x; 1.5001x over previous
"""FeaStConv dual-branch GNN message passing on 8 Trainium2 NeuronCores.

Sharding: branch v on cores 0-3, branch f on cores 4-7; each core owns a
12500-node destination range. Host reorders edges by destination block
(64 nodes), pre-gathers transposed source/dest features (bf16) plus a
block-local destination one-hot (fp8, exact 0/1). Device does all float
math. Scatter matmuls run one superchunk behind the projection matmuls so
the q*xjw multiply (split between Vector and Scalar-copy+GpSimd) has a
full superchunk of slack before its results are consumed.
"""
import sys, types
import numpy as np

sys.path.insert(0, '/opt/trn_rl_repo')

N = 50000
IN_CH = 64
HEADS = 4
OUT_CH = 32
P = 128
NPC = 12500           # nodes per core
BLK = 64              # dst nodes per block
NBLK = 196            # blocks per core (196*64 = 12544)
NPAD = NBLK * BLK
CH = 8               # tiles per chunk
NCHK = 4             # chunks per superchunk
SCT = CH * NCHK       # tiles per superchunk (32)
NCORES = 8
G_NUM = 1             # chunks with ci_g % G_DEN < G_NUM take the gpsimd path
G_DEN = 2


def _register_ntff_hook():
    import antenv
    if "antenv.axon_hooks" in sys.modules:
        return
    mod = types.ModuleType("antenv.axon_hooks")
    _h = [None]
    mod.set_axon_ntff_profile_hook = lambda h: _h.__setitem__(0, h)
    mod.get_axon_ntff_profile_hook = lambda: _h[0]
    sys.modules["antenv.axon_hooks"] = mod
    antenv.axon_hooks = mod
    if "/root/.axon_site" not in sys.path:
        sys.path.insert(0, "/root/.axon_site")
    try:
        from trn_agent_boot.trn_boot import _ntff_profile_via_ctypes
        mod.set_axon_ntff_profile_hook(_ntff_profile_via_ctypes('/opt/axon/libaxon_pjrt.so'))
    except Exception:
        pass


def _prep_core(x16, src, dst, lo):
    sel = (dst >= lo) & (dst < lo + NPC)
    s = src[sel]
    d = (dst[sel] - lo).astype(np.int64)
    order = np.argsort(d, kind='stable')
    s = s[order]
    d = d[order]
    blk = d >> 6
    cnt = np.bincount(blk, minlength=NBLK).astype(np.int64)
    deg = np.bincount(d, minlength=NPAD).astype(np.float32)
    return {"s": s, "d": d, "cnt": cnt, "deg": deg}


def _build_core_arrays(ml, core, TPB, base, NT):
    import ml_dtypes
    x16, W, U, c, b = core["x16"], core["W"], core["U"], core["c"], core["b"]
    s, d, cnt = core["g"]["s"], core["g"]["d"], core["g"]["cnt"]
    E_pad = NT * P
    srcg = np.zeros(E_pad, np.int64)
    dstg = np.zeros(E_pad, np.int64)
    dl = np.full(E_pad, -1.0, np.float32)
    cstart = np.concatenate([[0], np.cumsum(cnt)])
    for k in range(NBLK):
        n_k = int(cnt[k])
        if n_k == 0:
            continue
        p0 = base[k] * P
        srcg[p0:p0 + n_k] = s[cstart[k]:cstart[k] + n_k]
        dstg[p0:p0 + n_k] = d[cstart[k]:cstart[k] + n_k] + core["lo"]
        dl[p0:p0 + n_k] = (d[cstart[k]:cstart[k] + n_k] - BLK * k).astype(np.float32)
    xsd = np.empty((P, E_pad), ml_dtypes.bfloat16)
    xsd[:IN_CH, :] = x16[srcg].T
    xsd[IN_CH:, :] = x16[dstg].T
    dlr = dl.reshape(NT, P)
    oh = (dlr[:, :, None] == np.arange(BLK, dtype=np.float32)[None, None, :])
    ohm = np.ascontiguousarray(
        oh.transpose(1, 0, 2).reshape(P, NT * BLK)).astype(ml_dtypes.float8_e4m3fn)
    Wcm = np.zeros((P, P), np.float32)
    Wcm[:IN_CH] = W
    UUc = np.concatenate([U, -U], axis=0)  # [128, 4]
    degp = np.ascontiguousarray(core["g"]["deg"].reshape(NBLK // 2, P).T)  # [128, 98]
    return {
        "xsd": xsd,
        "ohm": ohm,
        "wcm": Wcm.astype(ml_dtypes.bfloat16),
        "uuc": UUc.astype(ml_dtypes.bfloat16),
        "crep": np.tile(c[None, :], (P, 1)).astype(np.float32),
        "brep": np.tile(b[None, :], (P, 1)).astype(np.float32),
        "degp": degp.astype(np.float32),
    }


def _build_program(TPB, NT):
    import concourse.bass as bass
    import concourse.mybir as mybir
    import concourse.bacc as bacc
    from concourse.tile import TileContext

    dt = mybir.dt
    NSC = NT // SCT
    blk_of = np.repeat(np.arange(NBLK), TPB)
    t0 = np.concatenate([[0], np.cumsum(TPB)])

    nc = bacc.Bacc("TRN2", target_bir_lowering=False, debug=False, num_devices=NCORES)
    xsd_d = nc.dram_tensor("xsd", [P, NT * P], dt.bfloat16, kind="ExternalInput").ap()
    ohm_d = nc.dram_tensor("ohm", [P, NT * BLK], dt.float8e4, kind="ExternalInput").ap()
    wcm_d = nc.dram_tensor("wcm", [P, P], dt.bfloat16, kind="ExternalInput").ap()
    uuc_d = nc.dram_tensor("uuc", [P, 4], dt.bfloat16, kind="ExternalInput").ap()
    crep_d = nc.dram_tensor("crep", [P, 4], dt.float32, kind="ExternalInput").ap()
    brep_d = nc.dram_tensor("brep", [P, OUT_CH], dt.float32, kind="ExternalInput").ap()
    degp_d = nc.dram_tensor("degp", [P, NBLK // 2], dt.float32, kind="ExternalInput").ap()
    out_d = nc.dram_tensor("out", [NPAD, OUT_CH], dt.float32, kind="ExternalOutput").ap()

    def APn(t, dims, off=0):
        a = t[:]
        return bass.AP(a.tensor, a.offset + off, [a.ap[0]] + dims)

    with TileContext(nc) as tc:
        with tc.tile_pool(name="const", bufs=1) as cp, \
             tc.tile_pool(name="mega", bufs=4) as mp, \
             tc.tile_pool(name="ohp", bufs=4) as op_, \
             tc.tile_pool(name="work", bufs=10) as wp, \
             tc.tile_pool(name="cpb", bufs=3) as cb, \
             tc.tile_pool(name="qp", bufs=2) as qp, \
             tc.tile_pool(name="fin", bufs=2) as fp, \
             tc.tile_pool(name="finacc", bufs=1) as fap, \
             tc.tile_pool(name="psA", bufs=2, space="PSUM") as psA, \
             tc.tile_pool(name="psU", bufs=2, space="PSUM") as psU, \
             tc.tile_pool(name="psG", bufs=2, space="PSUM") as psG:

            wcm = cp.tile([P, P], dt.bfloat16)
            uuc = cp.tile([P, 4], dt.bfloat16)
            crep = cp.tile([P, 4], dt.float32)
            brep = cp.tile([P, OUT_CH], dt.float32)
            degp = cp.tile([P, NBLK // 2], dt.float32)
            expc = cp.tile([P, 4], dt.float32)
            nc.sync.dma_start(out=wcm[:], in_=wcm_d[:])
            nc.sync.dma_start(out=uuc[:], in_=uuc_d[:])
            nc.sync.dma_start(out=crep[:], in_=crep_d[:])
            nc.sync.dma_start(out=brep[:], in_=brep_d[:])
            nc.sync.dma_start(out=degp[:], in_=degp_d[:])
            nc.scalar.activation(expc[:], crep[:], mybir.ActivationFunctionType.Exp)

            fin = fap.tile([P, (NBLK // 2) * P], dt.float32)

            NH = NBLK // 2
            FIN_BOUNDS = [25, 50, 75, NH]

            def emit_finale(g0, g1):
                ng = g1 - g0
                hs = fp.tile([P, ng * OUT_CH], dt.float32, tag="hs", name="hs")
                h2 = fp.tile([P, ng * OUT_CH], dt.float32, tag="h2", name="h2")
                nc.vector.tensor_tensor(
                    out=APn(hs, [[32, ng], [1, 32]]),
                    in0=APn(fin, [[P, ng], [1, 32]], off=g0 * P),
                    in1=APn(fin, [[P, ng], [1, 32]], off=g0 * P + 32),
                    op=mybir.AluOpType.add)
                nc.vector.tensor_tensor(
                    out=APn(h2, [[32, ng], [1, 32]]),
                    in0=APn(fin, [[P, ng], [1, 32]], off=g0 * P + 64),
                    in1=APn(fin, [[P, ng], [1, 32]], off=g0 * P + 96),
                    op=mybir.AluOpType.add)
                nc.vector.tensor_tensor(
                    out=APn(hs, [[32, ng], [1, 32]]),
                    in0=APn(hs, [[32, ng], [1, 32]]),
                    in1=APn(h2, [[32, ng], [1, 32]]),
                    op=mybir.AluOpType.add)
                dmx = fp.tile([P, ng], dt.float32, tag="dmx", name="dmx")
                nc.vector.tensor_scalar(out=dmx[:], in0=degp[:, g0:g1],
                                        scalar1=1.0, scalar2=None,
                                        op0=mybir.AluOpType.max)
                drc = fp.tile([P, ng], dt.float32, tag="drc", name="drc")
                nc.vector.reciprocal(out=drc[:], in_=dmx[:])
                o1 = fp.tile([P, ng * OUT_CH], dt.float32, tag="o1", name="o1")
                nc.vector.tensor_tensor(
                    out=APn(o1, [[32, ng], [1, 32]]),
                    in0=APn(hs, [[32, ng], [1, 32]]),
                    in1=APn(drc, [[1, ng], [0, 32]]),
                    op=mybir.AluOpType.mult)
                nc.vector.tensor_tensor(
                    out=APn(o1, [[32, ng], [1, 32]]),
                    in0=APn(o1, [[32, ng], [1, 32]]),
                    in1=APn(brep, [[0, ng], [1, 32]]),
                    op=mybir.AluOpType.add)
                o2 = fp.tile([P, ng * OUT_CH], dt.float32, tag="o2", name="o2")
                nc.vector.tensor_scalar(out=o2[:], in0=o1[:], scalar1=0.2,
                                        scalar2=None, op0=mybir.AluOpType.mult)
                nc.vector.tensor_tensor(out=o1[:], in0=o1[:], in1=o2[:],
                                        op=mybir.AluOpType.max)
                out_ap = bass.AP(out_d.tensor, out_d.offset + g0 * P * OUT_CH,
                                 [[OUT_CH, P], [P * OUT_CH, ng], [1, OUT_CH]])
                nc.sync.dma_start(out=out_ap, in_=APn(o1, [[32, ng], [1, 32]]))

            state = {"acc": None}

            def emit_scatter(sc, stgs, ohm):
                for ci in range(NCHK):
                    stg = stgs[ci]
                    for i in range(CH):
                        t = sc * SCT + ci * CH + i
                        k = int(blk_of[t])
                        if k % 2 == 0 and t == t0[k]:
                            state["acc"] = psG.tile([P, P], dt.float32, tag="acc", name="acc")
                        acc = state["acc"]
                        half = (k % 2) * BLK
                        nc.tensor.matmul(
                            out=acc[half:half + BLK, :],
                            lhsT=ohm[:, (ci * CH + i) * BLK:(ci * CH + i + 1) * BLK],
                            rhs=stg[:, i * P:(i + 1) * P],
                            start=(t == t0[k]), stop=(t == t0[k + 1] - 1))
                        if k % 2 == 1 and t == t0[k + 1] - 1:
                            m = k // 2
                            nc.scalar.copy(out=fin[:, m * P:(m + 1) * P], in_=acc[:])
                            if (m + 1) in FIN_BOUNDS:
                                emit_finale(FIN_BOUNDS[FIN_BOUNDS.index(m + 1) - 1]
                                            if FIN_BOUNDS.index(m + 1) > 0 else 0,
                                            m + 1)

            prev = None
            for sc in range(NSC):
                xm = mp.tile([P, SCT * P], dt.bfloat16, tag="xm", name="xm")
                nc.sync.dma_start(out=xm[:], in_=xsd_d[:, sc * SCT * P:(sc + 1) * SCT * P])
                ohm = op_.tile([P, SCT * BLK], dt.float8e4, tag="ohm", name="ohm")
                nc.sync.dma_start(out=ohm[:], in_=ohm_d[:, sc * SCT * BLK:(sc + 1) * SCT * BLK])
                pU = psU.tile([P, SCT * 4], dt.float32, tag="pU", name="pU")
                qe = qp.tile([P, SCT * 4], dt.float32, tag="qe", name="qe")
                qb = qp.tile([P, SCT * 4], dt.float32, tag="qb", name="qb")
                qd = qp.tile([P, SCT * 2], dt.float32, tag="qd", name="qd")
                den = qp.tile([P, SCT], dt.float32, tag="den", name="den")
                rec = qp.tile([P, SCT], dt.float32, tag="rec", name="rec")

                chunk_data = []
                for ci in range(NCHK):
                    pA = psA.tile([P, CH * P], dt.float32, tag="pA", name="pA")
                    for i in range(CH):
                        lhs = xm[:, (ci * CH + i) * P:(ci * CH + i + 1) * P]
                        nc.tensor.matmul(out=pA[:, i * P:(i + 1) * P], lhsT=lhs,
                                         rhs=wcm[:], start=True, stop=True)
                        nc.tensor.matmul(out=pU[:, (ci * CH + i) * 4:(ci * CH + i + 1) * 4],
                                         lhsT=lhs, rhs=uuc[:], start=True, stop=True)
                    chunk_data.append(pA)

                # softmax over the whole superchunk
                nc.scalar.activation(qe[:], pU[:], mybir.ActivationFunctionType.Exp)
                nc.vector.tensor_tensor(
                    out=APn(qb, [[4, SCT], [1, 4]]),
                    in0=APn(qe, [[4, SCT], [1, 4]]),
                    in1=APn(expc, [[0, SCT], [1, 4]]),
                    op=mybir.AluOpType.mult)
                nc.vector.tensor_tensor(
                    out=APn(qd, [[2, SCT], [1, 2]]),
                    in0=APn(qb, [[4, SCT], [1, 2]]),
                    in1=APn(qb, [[4, SCT], [1, 2]], off=2),
                    op=mybir.AluOpType.add)
                nc.vector.tensor_tensor(
                    out=APn(den, [[1, SCT], [1, 1]]),
                    in0=APn(qd, [[2, SCT], [1, 1]]),
                    in1=APn(qd, [[2, SCT], [1, 1]], off=1),
                    op=mybir.AluOpType.add)
                nc.vector.reciprocal(out=rec[:], in_=den[:])
                nc.vector.tensor_tensor(
                    out=APn(qe, [[4, SCT], [1, 4]]),
                    in0=APn(qb, [[4, SCT], [1, 4]]),
                    in1=APn(rec, [[1, SCT], [0, 4]]),
                    op=mybir.AluOpType.mult)

                cur = []
                for ci in range(NCHK):
                    pA = chunk_data[ci]
                    ci_g = sc * NCHK + ci
                    stg = wp.tile([P, CH * P], dt.bfloat16, tag="stg", name="stg")
                    if ci_g % G_DEN < G_NUM:
                        sA = cb.tile([P, CH * P], dt.bfloat16, tag="sA", name="sA")
                        nc.scalar.copy(out=sA[:], in_=pA[:])
                        nc.gpsimd.tensor_tensor(
                            out=APn(stg, [[P, CH], [32, 4], [1, 32]]),
                            in0=APn(sA, [[P, CH], [32, 4], [1, 32]]),
                            in1=APn(qe, [[4, CH], [1, 4], [0, 32]], off=ci * CH * 4),
                            op=mybir.AluOpType.mult)
                    else:
                        nc.vector.tensor_tensor(
                            out=APn(stg, [[P, CH], [32, 4], [1, 32]]),
                            in0=APn(pA, [[P, CH], [32, 4], [1, 32]]),
                            in1=APn(qe, [[4, CH], [1, 4], [0, 32]], off=ci * CH * 4),
                            op=mybir.AluOpType.mult)
                    cur.append(stg)

                if prev is not None:
                    emit_scatter(sc - 1, prev[0], prev[1])
                prev = (cur, ohm)
            emit_scatter(NSC - 1, prev[0], prev[1])
    nc.compile()
    return nc


def kernel(x_v, edge_index_v, x_f, edge_index_f, Wv, Uv, cv, bv, Wf, Uf, cf, bf):
    _register_ntff_hook()
    import ml_dtypes
    from concourse import bass_utils

    x_v = np.asarray(x_v, np.float32)
    x_f = np.asarray(x_f, np.float32)
    cores = []
    for bi, (x, ei, W, U, c, b) in enumerate([
            (x_v, edge_index_v, Wv, Uv, cv, bv),
            (x_f, edge_index_f, Wf, Uf, cf, bf)]):
        ei = np.asarray(ei)
        s0, d0 = ei[0].astype(np.int64), ei[1].astype(np.int64)
        m = s0 != d0
        loops = np.arange(N, dtype=np.int64)
        src = np.concatenate([s0[m], loops])
        dst = np.concatenate([d0[m], loops])
        x16 = x.astype(ml_dtypes.bfloat16)
        for j in range(4):
            lo = j * NPC
            cores.append({
                "x16": x16, "W": np.asarray(W, np.float32),
                "U": np.asarray(U, np.float32), "c": np.asarray(c, np.float32),
                "b": np.asarray(b, np.float32), "lo": lo,
                "g": _prep_core(x16, src, dst, lo),
            })

    tn = np.stack([np.ceil(c["g"]["cnt"] / P).astype(np.int64) for c in cores])
    TPB = tn.max(axis=0)
    TPB = np.maximum(TPB, 1)
    NT = int(TPB.sum())
    pad = (-NT) % SCT
    TPB[NBLK - 1] += pad
    NT += pad
    base = np.concatenate([[0], np.cumsum(TPB)])[:-1]

    in_maps = []
    for c in cores:
        arrs = _build_core_arrays(None, c, TPB, base, NT)
        in_maps.append(arrs)

    nc = _build_program(TPB, NT)
    res = bass_utils.run_bass_kernel_spmd(
        nc, in_maps, core_ids=list(range(NCORES)),
        trace=bool(int(__import__("os").environ.get("KERNEL_TRACE", "0"))))
    kernel.last_result = res
    out_v = np.concatenate([res.results[j]["out"][:NPC] for j in range(4)])
    out_f = np.concatenate([res.results[4 + j]["out"][:NPC] for j in range(4)])
    return out_v, out_f


# revision 24
# speedup vs baseline: 1.7407x; 1.0224x over previous
"""FeaStConv dual-branch GNN message passing on 8 Trainium2 NeuronCores.

Sharding: branch v on cores 0-3, branch f on cores 4-7; each core owns a
12500-node destination range. Host reorders edges by destination block
(64 nodes), pre-gathers transposed source/dest features (bf16) plus a
block-local destination one-hot (fp8, exact 0/1). Device does all float
math. Scatter matmuls run one superchunk behind the projection matmuls so
the q*xjw multiply (split between Vector and Scalar-copy+GpSimd) has a
full superchunk of slack before its results are consumed.
"""
import sys, types
import numpy as np

sys.path.insert(0, '/opt/trn_rl_repo')

N = 50000
IN_CH = 64
HEADS = 4
OUT_CH = 32
P = 128
NPC = 12500           # nodes per core
BLK = 64              # dst nodes per block
NBLK = 196            # blocks per core (196*64 = 12544)
NPAD = NBLK * BLK
CH = 8               # tiles per chunk
NCHK = 4             # chunks per superchunk
SCT = CH * NCHK       # tiles per superchunk (32)
NCORES = 8
G_NUM = 1             # chunks with ci_g % G_DEN < G_NUM take the gpsimd path
G_DEN = 2


def _register_ntff_hook():
    import antenv
    if "antenv.axon_hooks" in sys.modules:
        return
    mod = types.ModuleType("antenv.axon_hooks")
    _h = [None]
    mod.set_axon_ntff_profile_hook = lambda h: _h.__setitem__(0, h)
    mod.get_axon_ntff_profile_hook = lambda: _h[0]
    sys.modules["antenv.axon_hooks"] = mod
    antenv.axon_hooks = mod
    if "/root/.axon_site" not in sys.path:
        sys.path.insert(0, "/root/.axon_site")
    try:
        from trn_agent_boot.trn_boot import _ntff_profile_via_ctypes
        mod.set_axon_ntff_profile_hook(_ntff_profile_via_ctypes('/opt/axon/libaxon_pjrt.so'))
    except Exception:
        pass


def _prep_core(x16, src, dst, lo):
    sel = (dst >= lo) & (dst < lo + NPC)
    s = src[sel]
    d = (dst[sel] - lo).astype(np.int64)
    order = np.argsort(d, kind='stable')
    s = s[order]
    d = d[order]
    blk = d >> 6
    cnt = np.bincount(blk, minlength=NBLK).astype(np.int64)
    deg = np.bincount(d, minlength=NPAD).astype(np.float32)
    return {"s": s, "d": d, "cnt": cnt, "deg": deg}


def _build_core_arrays(ml, core, TPB, base, NT):
    import ml_dtypes
    x16, W, U, c, b = core["x16"], core["W"], core["U"], core["c"], core["b"]
    s, d, cnt = core["g"]["s"], core["g"]["d"], core["g"]["cnt"]
    E_pad = NT * P
    srcg = np.zeros(E_pad, np.int64)
    dstg = np.zeros(E_pad, np.int64)
    dl = np.full(E_pad, -1.0, np.float32)
    cstart = np.concatenate([[0], np.cumsum(cnt)])
    for k in range(NBLK):
        n_k = int(cnt[k])
        if n_k == 0:
            continue
        p0 = base[k] * P
        srcg[p0:p0 + n_k] = s[cstart[k]:cstart[k] + n_k]
        dstg[p0:p0 + n_k] = d[cstart[k]:cstart[k] + n_k] + core["lo"]
        dl[p0:p0 + n_k] = (d[cstart[k]:cstart[k] + n_k] - BLK * k).astype(np.float32)
    xsd = np.empty((P, E_pad), ml_dtypes.bfloat16)
    xsd[:IN_CH, :] = x16[srcg].T
    xsd[IN_CH:, :] = x16[dstg].T
    dlr = dl.reshape(NT, P)
    oh = (dlr[:, :, None] == np.arange(BLK, dtype=np.float32)[None, None, :])
    ohm = np.ascontiguousarray(
        oh.transpose(1, 0, 2).reshape(P, NT * BLK)).astype(ml_dtypes.float8_e4m3fn)
    Wcm = np.zeros((P, P), np.float32)
    Wcm[:IN_CH] = W
    UUc = np.concatenate([U, -U], axis=0)  # [128, 4]
    degp = np.ascontiguousarray(core["g"]["deg"].reshape(NBLK // 2, P).T)  # [128, 98]
    return {
        "xsd": xsd,
        "ohm": ohm,
        "wcm": Wcm.astype(ml_dtypes.bfloat16),
        "uuc": UUc.astype(ml_dtypes.bfloat16),
        "crep": np.tile(c[None, :], (P, 1)).astype(np.float32),
        "brep": np.tile(b[None, :], (P, 1)).astype(np.float32),
        "degp": degp.astype(np.float32),
    }


def _build_program(TPB, NT):
    import concourse.bass as bass
    import concourse.mybir as mybir
    import concourse.bacc as bacc
    from concourse.tile import TileContext

    dt = mybir.dt
    NSC = NT // SCT
    blk_of = np.repeat(np.arange(NBLK), TPB)
    t0 = np.concatenate([[0], np.cumsum(TPB)])

    nc = bacc.Bacc("TRN2", target_bir_lowering=False, debug=False, num_devices=NCORES)
    xsd_d = nc.dram_tensor("xsd", [P, NT * P], dt.bfloat16, kind="ExternalInput").ap()
    ohm_d = nc.dram_tensor("ohm", [P, NT * BLK], dt.float8e4, kind="ExternalInput").ap()
    wcm_d = nc.dram_tensor("wcm", [P, P], dt.bfloat16, kind="ExternalInput").ap()
    uuc_d = nc.dram_tensor("uuc", [P, 4], dt.bfloat16, kind="ExternalInput").ap()
    crep_d = nc.dram_tensor("crep", [P, 4], dt.float32, kind="ExternalInput").ap()
    brep_d = nc.dram_tensor("brep", [P, OUT_CH], dt.float32, kind="ExternalInput").ap()
    degp_d = nc.dram_tensor("degp", [P, NBLK // 2], dt.float32, kind="ExternalInput").ap()
    out_d = nc.dram_tensor("out", [NPAD, OUT_CH], dt.float32, kind="ExternalOutput").ap()

    def APn(t, dims, off=0):
        a = t[:]
        return bass.AP(a.tensor, a.offset + off, [a.ap[0]] + dims)

    with TileContext(nc) as tc:
        with tc.tile_pool(name="const", bufs=1) as cp, \
             tc.tile_pool(name="mega", bufs=4) as mp, \
             tc.tile_pool(name="ohp", bufs=4) as op_, \
             tc.tile_pool(name="work", bufs=12) as wp, \
             tc.tile_pool(name="cpb", bufs=4) as cb, \
             tc.tile_pool(name="qp", bufs=2) as qp, \
             tc.tile_pool(name="fin", bufs=2) as fp, \
             tc.tile_pool(name="finacc", bufs=1) as fap, \
             tc.tile_pool(name="psA", bufs=2, space="PSUM") as psA, \
             tc.tile_pool(name="psU", bufs=2, space="PSUM") as psU, \
             tc.tile_pool(name="psG", bufs=2, space="PSUM") as psG:

            wcm = cp.tile([P, P], dt.bfloat16)
            uuc = cp.tile([P, 4], dt.bfloat16)
            crep = cp.tile([P, 4], dt.float32)
            brep = cp.tile([P, OUT_CH], dt.float32)
            degp = cp.tile([P, NBLK // 2], dt.float32)
            expc = cp.tile([P, 4], dt.float32)
            nc.sync.dma_start(out=wcm[:], in_=wcm_d[:])
            nc.sync.dma_start(out=uuc[:], in_=uuc_d[:])
            nc.sync.dma_start(out=crep[:], in_=crep_d[:])
            nc.sync.dma_start(out=brep[:], in_=brep_d[:])
            nc.sync.dma_start(out=degp[:], in_=degp_d[:])
            nc.scalar.activation(expc[:], crep[:], mybir.ActivationFunctionType.Exp)

            fin = fap.tile([P, (NBLK // 2) * P], dt.float32)

            NH = NBLK // 2
            FIN_BOUNDS = [25, 50, 75, 90, NH]

            def emit_finale(g0, g1):
                ng = g1 - g0
                hs = fp.tile([P, ng * OUT_CH], dt.float32, tag="hs", name="hs")
                h2 = fp.tile([P, ng * OUT_CH], dt.float32, tag="h2", name="h2")
                nc.vector.tensor_tensor(
                    out=APn(hs, [[32, ng], [1, 32]]),
                    in0=APn(fin, [[P, ng], [1, 32]], off=g0 * P),
                    in1=APn(fin, [[P, ng], [1, 32]], off=g0 * P + 32),
                    op=mybir.AluOpType.add)
                nc.vector.tensor_tensor(
                    out=APn(h2, [[32, ng], [1, 32]]),
                    in0=APn(fin, [[P, ng], [1, 32]], off=g0 * P + 64),
                    in1=APn(fin, [[P, ng], [1, 32]], off=g0 * P + 96),
                    op=mybir.AluOpType.add)
                nc.vector.tensor_tensor(
                    out=APn(hs, [[32, ng], [1, 32]]),
                    in0=APn(hs, [[32, ng], [1, 32]]),
                    in1=APn(h2, [[32, ng], [1, 32]]),
                    op=mybir.AluOpType.add)
                dmx = fp.tile([P, ng], dt.float32, tag="dmx", name="dmx")
                nc.vector.tensor_scalar(out=dmx[:], in0=degp[:, g0:g1],
                                        scalar1=1.0, scalar2=None,
                                        op0=mybir.AluOpType.max)
                drc = fp.tile([P, ng], dt.float32, tag="drc", name="drc")
                nc.vector.reciprocal(out=drc[:], in_=dmx[:])
                o1 = fp.tile([P, ng * OUT_CH], dt.float32, tag="o1", name="o1")
                nc.vector.tensor_tensor(
                    out=APn(o1, [[32, ng], [1, 32]]),
                    in0=APn(hs, [[32, ng], [1, 32]]),
                    in1=APn(drc, [[1, ng], [0, 32]]),
                    op=mybir.AluOpType.mult)
                nc.vector.tensor_tensor(
                    out=APn(o1, [[32, ng], [1, 32]]),
                    in0=APn(o1, [[32, ng], [1, 32]]),
                    in1=APn(brep, [[0, ng], [1, 32]]),
                    op=mybir.AluOpType.add)
                o2 = fp.tile([P, ng * OUT_CH], dt.float32, tag="o2", name="o2")
                nc.vector.tensor_scalar(out=o2[:], in0=o1[:], scalar1=0.2,
                                        scalar2=None, op0=mybir.AluOpType.mult)
                nc.vector.tensor_tensor(out=o1[:], in0=o1[:], in1=o2[:],
                                        op=mybir.AluOpType.max)
                out_ap = bass.AP(out_d.tensor, out_d.offset + g0 * P * OUT_CH,
                                 [[OUT_CH, P], [P * OUT_CH, ng], [1, OUT_CH]])
                nc.sync.dma_start(out=out_ap, in_=APn(o1, [[32, ng], [1, 32]]))

            state = {"acc": None}

            def emit_scatter(sc, stgs, ohm, cis):
                for ci in cis:
                    stg = stgs[ci]
                    for i in range(CH):
                        t = sc * SCT + ci * CH + i
                        k = int(blk_of[t])
                        if k % 2 == 0 and t == t0[k]:
                            state["acc"] = psG.tile([P, P], dt.float32, tag="acc", name="acc")
                        acc = state["acc"]
                        half = (k % 2) * BLK
                        nc.tensor.matmul(
                            out=acc[half:half + BLK, :],
                            lhsT=ohm[:, (ci * CH + i) * BLK:(ci * CH + i + 1) * BLK],
                            rhs=stg[:, i * P:(i + 1) * P],
                            start=(t == t0[k]), stop=(t == t0[k + 1] - 1))
                        if k % 2 == 1 and t == t0[k + 1] - 1:
                            m = k // 2
                            nc.scalar.copy(out=fin[:, m * P:(m + 1) * P], in_=acc[:])
                            if (m + 1) in FIN_BOUNDS:
                                emit_finale(FIN_BOUNDS[FIN_BOUNDS.index(m + 1) - 1]
                                            if FIN_BOUNDS.index(m + 1) > 0 else 0,
                                            m + 1)

            prev = None
            for sc in range(NSC):
                xm = mp.tile([P, SCT * P], dt.bfloat16, tag="xm", name="xm")
                nc.sync.dma_start(out=xm[:], in_=xsd_d[:, sc * SCT * P:(sc + 1) * SCT * P])
                ohm = op_.tile([P, SCT * BLK], dt.float8e4, tag="ohm", name="ohm")
                nc.sync.dma_start(out=ohm[:], in_=ohm_d[:, sc * SCT * BLK:(sc + 1) * SCT * BLK])
                pU = psU.tile([P, SCT * 4], dt.float32, tag="pU", name="pU")
                qe = qp.tile([P, SCT * 4], dt.float32, tag="qe", name="qe")
                qb = qp.tile([P, SCT * 4], dt.float32, tag="qb", name="qb")
                qd = qp.tile([P, SCT * 2], dt.float32, tag="qd", name="qd")
                den = qp.tile([P, SCT], dt.float32, tag="den", name="den")
                rec = qp.tile([P, SCT], dt.float32, tag="rec", name="rec")

                chunk_data = []
                for ci in range(NCHK):
                    pA = psA.tile([P, CH * P], dt.float32, tag="pA", name="pA")
                    for i in range(CH):
                        lhs = xm[:, (ci * CH + i) * P:(ci * CH + i + 1) * P]
                        nc.tensor.matmul(out=pA[:, i * P:(i + 1) * P], lhsT=lhs,
                                         rhs=wcm[:], start=True, stop=True)
                        nc.tensor.matmul(out=pU[:, (ci * CH + i) * 4:(ci * CH + i + 1) * 4],
                                         lhsT=lhs, rhs=uuc[:], start=True, stop=True)
                    chunk_data.append(pA)
                    # interleave one previous-superchunk scatter chunk after each
                    # chunk's matmuls: guaranteed-ready fill work for the tensor
                    # queue while this superchunk's softmax/stg latency resolves
                    if prev is not None:
                        emit_scatter(sc - 1, prev[0], prev[1], [ci])

                # softmax over the whole superchunk
                nc.scalar.activation(qe[:], pU[:], mybir.ActivationFunctionType.Exp)
                nc.vector.tensor_tensor(
                    out=APn(qb, [[4, SCT], [1, 4]]),
                    in0=APn(qe, [[4, SCT], [1, 4]]),
                    in1=APn(expc, [[0, SCT], [1, 4]]),
                    op=mybir.AluOpType.mult)
                nc.vector.tensor_tensor(
                    out=APn(qd, [[2, SCT], [1, 2]]),
                    in0=APn(qb, [[4, SCT], [1, 2]]),
                    in1=APn(qb, [[4, SCT], [1, 2]], off=2),
                    op=mybir.AluOpType.add)
                nc.vector.tensor_tensor(
                    out=APn(den, [[1, SCT], [1, 1]]),
                    in0=APn(qd, [[2, SCT], [1, 1]]),
                    in1=APn(qd, [[2, SCT], [1, 1]], off=1),
                    op=mybir.AluOpType.add)
                nc.vector.reciprocal(out=rec[:], in_=den[:])
                nc.vector.tensor_tensor(
                    out=APn(qe, [[4, SCT], [1, 4]]),
                    in0=APn(qb, [[4, SCT], [1, 4]]),
                    in1=APn(rec, [[1, SCT], [0, 4]]),
                    op=mybir.AluOpType.mult)

                cur = []
                for ci in range(NCHK):
                    pA = chunk_data[ci]
                    ci_g = sc * NCHK + ci
                    stg = wp.tile([P, CH * P], dt.bfloat16, tag="stg", name="stg")
                    if ci_g % G_DEN < G_NUM:
                        sA = cb.tile([P, CH * P], dt.bfloat16, tag="sA", name="sA")
                        nc.scalar.copy(out=sA[:], in_=pA[:])
                        nc.gpsimd.tensor_tensor(
                            out=APn(stg, [[P, CH], [32, 4], [1, 32]]),
                            in0=APn(sA, [[P, CH], [32, 4], [1, 32]]),
                            in1=APn(qe, [[4, CH], [1, 4], [0, 32]], off=ci * CH * 4),
                            op=mybir.AluOpType.mult)
                    else:
                        nc.vector.tensor_tensor(
                            out=APn(stg, [[P, CH], [32, 4], [1, 32]]),
                            in0=APn(pA, [[P, CH], [32, 4], [1, 32]]),
                            in1=APn(qe, [[4, CH], [1, 4], [0, 32]], off=ci * CH * 4),
                            op=mybir.AluOpType.mult)
                    cur.append(stg)

                prev = (cur, ohm)
            emit_scatter(NSC - 1, prev[0], prev[1], range(NCHK))
    nc.compile()
    return nc


def kernel(x_v, edge_index_v, x_f, edge_index_f, Wv, Uv, cv, bv, Wf, Uf, cf, bf):
    _register_ntff_hook()
    import ml_dtypes
    from concourse import bass_utils

    x_v = np.asarray(x_v, np.float32)
    x_f = np.asarray(x_f, np.float32)
    cores = []
    for bi, (x, ei, W, U, c, b) in enumerate([
            (x_v, edge_index_v, Wv, Uv, cv, bv),
            (x_f, edge_index_f, Wf, Uf, cf, bf)]):
        ei = np.asarray(ei)
        s0, d0 = ei[0].astype(np.int64), ei[1].astype(np.int64)
        m = s0 != d0
        loops = np.arange(N, dtype=np.int64)
        src = np.concatenate([s0[m], loops])
        dst = np.concatenate([d0[m], loops])
        x16 = x.astype(ml_dtypes.bfloat16)
        for j in range(4):
            lo = j * NPC
            cores.append({
                "x16": x16, "W": np.asarray(W, np.float32),
                "U": np.asarray(U, np.float32), "c": np.asarray(c, np.float32),
                "b": np.asarray(b, np.float32), "lo": lo,
                "g": _prep_core(x16, src, dst, lo),
            })

    tn = np.stack([np.ceil(c["g"]["cnt"] / P).astype(np.int64) for c in cores])
    TPB = tn.max(axis=0)
    TPB = np.maximum(TPB, 1)
    NT = int(TPB.sum())
    pad = (-NT) % SCT
    TPB[NBLK - 1] += pad
    NT += pad
    base = np.concatenate([[0], np.cumsum(TPB)])[:-1]

    in_maps = []
    for c in cores:
        arrs = _build_core_arrays(None, c, TPB, base, NT)
        in_maps.append(arrs)

    nc = _build_program(TPB, NT)
    res = bass_utils.run_bass_kernel_spmd(
        nc, in_maps, core_ids=list(range(NCORES)),
        trace=bool(int(__import__("os").environ.get("KERNEL_TRACE", "0"))))
    kernel.last_result = res
    out_v = np.concatenate([res.results[j]["out"][:NPC] for j in range(4)])
    out_f = np.concatenate([res.results[4 + j]["out"][:NPC] for j in range(4)])
    return out_v, out_f


# revision 25
# speedup vs baseline: 1.7558x; 1.0087x over previous
"""FeaStConv dual-branch GNN message passing on 8 Trainium2 NeuronCores.

Sharding: branch v on cores 0-3, branch f on cores 4-7; each core owns a
12500-node destination range. Host reorders edges by destination block
(64 nodes), pre-gathers transposed source/dest features (bf16) plus a
block-local destination one-hot (fp8, exact 0/1). Device does all float
math. Scatter matmuls run one superchunk behind the projection matmuls so
the q*xjw multiply (split between Vector and Scalar-copy+GpSimd) has a
full superchunk of slack before its results are consumed.
"""
import sys, types
import numpy as np

sys.path.insert(0, '/opt/trn_rl_repo')

N = 50000
IN_CH = 64
HEADS = 4
OUT_CH = 32
P = 128
NPC = 12500           # nodes per core
BLK = 64              # dst nodes per block
NBLK = 196            # blocks per core (196*64 = 12544)
NPAD = NBLK * BLK
CH = 8               # tiles per chunk
NCHK = 4             # chunks per superchunk
SCT = CH * NCHK       # tiles per superchunk (32)
NCORES = 8
G_NUM = 1             # chunks with ci_g % G_DEN < G_NUM take the gpsimd path
G_DEN = 2


def _register_ntff_hook():
    import antenv
    if "antenv.axon_hooks" in sys.modules:
        return
    mod = types.ModuleType("antenv.axon_hooks")
    _h = [None]
    mod.set_axon_ntff_profile_hook = lambda h: _h.__setitem__(0, h)
    mod.get_axon_ntff_profile_hook = lambda: _h[0]
    sys.modules["antenv.axon_hooks"] = mod
    antenv.axon_hooks = mod
    if "/root/.axon_site" not in sys.path:
        sys.path.insert(0, "/root/.axon_site")
    try:
        from trn_agent_boot.trn_boot import _ntff_profile_via_ctypes
        mod.set_axon_ntff_profile_hook(_ntff_profile_via_ctypes('/opt/axon/libaxon_pjrt.so'))
    except Exception:
        pass


def _prep_core(x16, src, dst, lo):
    sel = (dst >= lo) & (dst < lo + NPC)
    s = src[sel]
    d = (dst[sel] - lo).astype(np.int64)
    order = np.argsort(d, kind='stable')
    s = s[order]
    d = d[order]
    blk = d >> 6
    cnt = np.bincount(blk, minlength=NBLK).astype(np.int64)
    deg = np.bincount(d, minlength=NPAD).astype(np.float32)
    return {"s": s, "d": d, "cnt": cnt, "deg": deg}


def _build_core_arrays(ml, core, TPB, base, NT):
    import ml_dtypes
    x16, W, U, c, b = core["x16"], core["W"], core["U"], core["c"], core["b"]
    s, d, cnt = core["g"]["s"], core["g"]["d"], core["g"]["cnt"]
    E_pad = NT * P
    srcg = np.zeros(E_pad, np.int64)
    dstg = np.zeros(E_pad, np.int64)
    dl = np.full(E_pad, -1.0, np.float32)
    cstart = np.concatenate([[0], np.cumsum(cnt)])
    for k in range(NBLK):
        n_k = int(cnt[k])
        if n_k == 0:
            continue
        p0 = base[k] * P
        srcg[p0:p0 + n_k] = s[cstart[k]:cstart[k] + n_k]
        dstg[p0:p0 + n_k] = d[cstart[k]:cstart[k] + n_k] + core["lo"]
        dl[p0:p0 + n_k] = (d[cstart[k]:cstart[k] + n_k] - BLK * k).astype(np.float32)
    xsd = np.empty((P, E_pad), ml_dtypes.bfloat16)
    xsd[:IN_CH, :] = x16[srcg].T
    xsd[IN_CH:, :] = x16[dstg].T
    dlr = dl.reshape(NT, P)
    oh = (dlr[:, :, None] == np.arange(BLK, dtype=np.float32)[None, None, :])
    ohm = np.ascontiguousarray(
        oh.transpose(1, 0, 2).reshape(P, NT * BLK)).astype(ml_dtypes.float8_e4m3fn)
    Wcm = np.zeros((P, P), np.float32)
    Wcm[:IN_CH] = W
    UUc = np.concatenate([U, -U], axis=0)  # [128, 4]
    degp = np.ascontiguousarray(core["g"]["deg"].reshape(NBLK // 2, P).T)  # [128, 98]
    return {
        "xsd": xsd,
        "ohm": ohm,
        "wcm": Wcm.astype(ml_dtypes.bfloat16),
        "uuc": UUc.astype(ml_dtypes.bfloat16),
        "crep": np.tile(c[None, :], (P, 1)).astype(np.float32),
        "brep": np.tile(b[None, :], (P, 1)).astype(np.float32),
        "degp": degp.astype(np.float32),
    }


def _build_program(TPB, NT):
    import concourse.bass as bass
    import concourse.mybir as mybir
    import concourse.bacc as bacc
    from concourse.tile import TileContext

    dt = mybir.dt
    NSC = NT // SCT
    blk_of = np.repeat(np.arange(NBLK), TPB)
    t0 = np.concatenate([[0], np.cumsum(TPB)])

    nc = bacc.Bacc("TRN2", target_bir_lowering=False, debug=False, num_devices=NCORES)
    xsd_d = nc.dram_tensor("xsd", [P, NT * P], dt.bfloat16, kind="ExternalInput").ap()
    ohm_d = nc.dram_tensor("ohm", [P, NT * BLK], dt.float8e4, kind="ExternalInput").ap()
    wcm_d = nc.dram_tensor("wcm", [P, P], dt.bfloat16, kind="ExternalInput").ap()
    uuc_d = nc.dram_tensor("uuc", [P, 4], dt.bfloat16, kind="ExternalInput").ap()
    crep_d = nc.dram_tensor("crep", [P, 4], dt.float32, kind="ExternalInput").ap()
    brep_d = nc.dram_tensor("brep", [P, OUT_CH], dt.float32, kind="ExternalInput").ap()
    degp_d = nc.dram_tensor("degp", [P, NBLK // 2], dt.float32, kind="ExternalInput").ap()
    out_d = nc.dram_tensor("out", [NPAD, OUT_CH], dt.float32, kind="ExternalOutput").ap()

    def APn(t, dims, off=0):
        a = t[:]
        return bass.AP(a.tensor, a.offset + off, [a.ap[0]] + dims)

    with TileContext(nc) as tc:
        with tc.tile_pool(name="const", bufs=1) as cp, \
             tc.tile_pool(name="mega", bufs=4) as mp, \
             tc.tile_pool(name="ohp", bufs=4) as op_, \
             tc.tile_pool(name="work", bufs=12) as wp, \
             tc.tile_pool(name="cpb", bufs=4) as cb, \
             tc.tile_pool(name="qp", bufs=2) as qp, \
             tc.tile_pool(name="fin", bufs=2) as fp, \
             tc.tile_pool(name="finacc", bufs=1) as fap, \
             tc.tile_pool(name="psA", bufs=2, space="PSUM") as psA, \
             tc.tile_pool(name="psU", bufs=2, space="PSUM") as psU, \
             tc.tile_pool(name="psG", bufs=2, space="PSUM") as psG:

            wcm = cp.tile([P, P], dt.bfloat16)
            uuc = cp.tile([P, 4], dt.bfloat16)
            crep = cp.tile([P, 4], dt.float32)
            brep = cp.tile([P, OUT_CH], dt.float32)
            degp = cp.tile([P, NBLK // 2], dt.float32)
            expc = cp.tile([P, 4], dt.float32)
            nc.sync.dma_start(out=wcm[:], in_=wcm_d[:])
            nc.sync.dma_start(out=uuc[:], in_=uuc_d[:])
            nc.sync.dma_start(out=crep[:], in_=crep_d[:])
            nc.sync.dma_start(out=brep[:], in_=brep_d[:])
            nc.sync.dma_start(out=degp[:], in_=degp_d[:])
            nc.scalar.activation(expc[:], crep[:], mybir.ActivationFunctionType.Exp)

            fin = fap.tile([P, (NBLK // 2) * P], dt.float32)

            NH = NBLK // 2
            FIN_BOUNDS = [25, 50, 75, 88, 94, NH]

            def emit_finale(g0, g1):
                ng = g1 - g0
                hs = fp.tile([P, ng * OUT_CH], dt.float32, tag="hs", name="hs")
                h2 = fp.tile([P, ng * OUT_CH], dt.float32, tag="h2", name="h2")
                nc.vector.tensor_tensor(
                    out=APn(hs, [[32, ng], [1, 32]]),
                    in0=APn(fin, [[P, ng], [1, 32]], off=g0 * P),
                    in1=APn(fin, [[P, ng], [1, 32]], off=g0 * P + 32),
                    op=mybir.AluOpType.add)
                nc.vector.tensor_tensor(
                    out=APn(h2, [[32, ng], [1, 32]]),
                    in0=APn(fin, [[P, ng], [1, 32]], off=g0 * P + 64),
                    in1=APn(fin, [[P, ng], [1, 32]], off=g0 * P + 96),
                    op=mybir.AluOpType.add)
                nc.vector.tensor_tensor(
                    out=APn(hs, [[32, ng], [1, 32]]),
                    in0=APn(hs, [[32, ng], [1, 32]]),
                    in1=APn(h2, [[32, ng], [1, 32]]),
                    op=mybir.AluOpType.add)
                dmx = fp.tile([P, ng], dt.float32, tag="dmx", name="dmx")
                nc.vector.tensor_scalar(out=dmx[:], in0=degp[:, g0:g1],
                                        scalar1=1.0, scalar2=None,
                                        op0=mybir.AluOpType.max)
                drc = fp.tile([P, ng], dt.float32, tag="drc", name="drc")
                nc.vector.reciprocal(out=drc[:], in_=dmx[:])
                o1 = fp.tile([P, ng * OUT_CH], dt.float32, tag="o1", name="o1")
                nc.vector.tensor_tensor(
                    out=APn(o1, [[32, ng], [1, 32]]),
                    in0=APn(hs, [[32, ng], [1, 32]]),
                    in1=APn(drc, [[1, ng], [0, 32]]),
                    op=mybir.AluOpType.mult)
                nc.vector.tensor_tensor(
                    out=APn(o1, [[32, ng], [1, 32]]),
                    in0=APn(o1, [[32, ng], [1, 32]]),
                    in1=APn(brep, [[0, ng], [1, 32]]),
                    op=mybir.AluOpType.add)
                o2 = fp.tile([P, ng * OUT_CH], dt.float32, tag="o2", name="o2")
                nc.vector.tensor_scalar(out=o2[:], in0=o1[:], scalar1=0.2,
                                        scalar2=None, op0=mybir.AluOpType.mult)
                nc.vector.tensor_tensor(out=o1[:], in0=o1[:], in1=o2[:],
                                        op=mybir.AluOpType.max)
                out_ap = bass.AP(out_d.tensor, out_d.offset + g0 * P * OUT_CH,
                                 [[OUT_CH, P], [P * OUT_CH, ng], [1, OUT_CH]])
                nc.sync.dma_start(out=out_ap, in_=APn(o1, [[32, ng], [1, 32]]))

            state = {"acc": None}

            def emit_scatter(sc, stgs, ohm, cis):
                for ci in cis:
                    stg = stgs[ci]
                    for i in range(CH):
                        t = sc * SCT + ci * CH + i
                        k = int(blk_of[t])
                        if k % 2 == 0 and t == t0[k]:
                            state["acc"] = psG.tile([P, P], dt.float32, tag="acc", name="acc")
                        acc = state["acc"]
                        half = (k % 2) * BLK
                        nc.tensor.matmul(
                            out=acc[half:half + BLK, :],
                            lhsT=ohm[:, (ci * CH + i) * BLK:(ci * CH + i + 1) * BLK],
                            rhs=stg[:, i * P:(i + 1) * P],
                            start=(t == t0[k]), stop=(t == t0[k + 1] - 1))
                        if k % 2 == 1 and t == t0[k + 1] - 1:
                            m = k // 2
                            nc.scalar.copy(out=fin[:, m * P:(m + 1) * P], in_=acc[:])
                            if (m + 1) in FIN_BOUNDS:
                                emit_finale(FIN_BOUNDS[FIN_BOUNDS.index(m + 1) - 1]
                                            if FIN_BOUNDS.index(m + 1) > 0 else 0,
                                            m + 1)

            prev = None
            for sc in range(NSC):
                xm = mp.tile([P, SCT * P], dt.bfloat16, tag="xm", name="xm")
                for dci in range(NCHK):
                    nc.sync.dma_start(
                        out=xm[:, dci * CH * P:(dci + 1) * CH * P],
                        in_=xsd_d[:, (sc * SCT + dci * CH) * P:(sc * SCT + (dci + 1) * CH) * P])
                ohm = op_.tile([P, SCT * BLK], dt.float8e4, tag="ohm", name="ohm")
                nc.sync.dma_start(out=ohm[:], in_=ohm_d[:, sc * SCT * BLK:(sc + 1) * SCT * BLK])
                pU = psU.tile([P, SCT * 4], dt.float32, tag="pU", name="pU")
                qe = qp.tile([P, SCT * 4], dt.float32, tag="qe", name="qe")
                qb = qp.tile([P, SCT * 4], dt.float32, tag="qb", name="qb")
                qd = qp.tile([P, SCT * 2], dt.float32, tag="qd", name="qd")
                den = qp.tile([P, SCT], dt.float32, tag="den", name="den")
                rec = qp.tile([P, SCT], dt.float32, tag="rec", name="rec")

                chunk_data = []
                for ci in range(NCHK):
                    pA = psA.tile([P, CH * P], dt.float32, tag="pA", name="pA")
                    for i in range(CH):
                        lhs = xm[:, (ci * CH + i) * P:(ci * CH + i + 1) * P]
                        nc.tensor.matmul(out=pA[:, i * P:(i + 1) * P], lhsT=lhs,
                                         rhs=wcm[:], start=True, stop=True)
                        nc.tensor.matmul(out=pU[:, (ci * CH + i) * 4:(ci * CH + i + 1) * 4],
                                         lhsT=lhs, rhs=uuc[:], start=True, stop=True)
                    chunk_data.append(pA)
                    # interleave one previous-superchunk scatter chunk after each
                    # chunk's matmuls: guaranteed-ready fill work for the tensor
                    # queue while this superchunk's softmax/stg latency resolves
                    if prev is not None:
                        emit_scatter(sc - 1, prev[0], prev[1], [ci])

                # softmax over the whole superchunk
                nc.scalar.activation(qe[:], pU[:], mybir.ActivationFunctionType.Exp)
                nc.vector.tensor_tensor(
                    out=APn(qb, [[4, SCT], [1, 4]]),
                    in0=APn(qe, [[4, SCT], [1, 4]]),
                    in1=APn(expc, [[0, SCT], [1, 4]]),
                    op=mybir.AluOpType.mult)
                nc.vector.tensor_tensor(
                    out=APn(qd, [[2, SCT], [1, 2]]),
                    in0=APn(qb, [[4, SCT], [1, 2]]),
                    in1=APn(qb, [[4, SCT], [1, 2]], off=2),
                    op=mybir.AluOpType.add)
                nc.vector.tensor_tensor(
                    out=APn(den, [[1, SCT], [1, 1]]),
                    in0=APn(qd, [[2, SCT], [1, 1]]),
                    in1=APn(qd, [[2, SCT], [1, 1]], off=1),
                    op=mybir.AluOpType.add)
                nc.vector.reciprocal(out=rec[:], in_=den[:])
                nc.vector.tensor_tensor(
                    out=APn(qe, [[4, SCT], [1, 4]]),
                    in0=APn(qb, [[4, SCT], [1, 4]]),
                    in1=APn(rec, [[1, SCT], [0, 4]]),
                    op=mybir.AluOpType.mult)

                cur = []
                for ci in range(NCHK):
                    pA = chunk_data[ci]
                    ci_g = sc * NCHK + ci
                    stg = wp.tile([P, CH * P], dt.bfloat16, tag="stg", name="stg")
                    if ci_g % G_DEN < G_NUM:
                        sA = cb.tile([P, CH * P], dt.bfloat16, tag="sA", name="sA")
                        nc.scalar.copy(out=sA[:], in_=pA[:])
                        nc.gpsimd.tensor_tensor(
                            out=APn(stg, [[P, CH], [32, 4], [1, 32]]),
                            in0=APn(sA, [[P, CH], [32, 4], [1, 32]]),
                            in1=APn(qe, [[4, CH], [1, 4], [0, 32]], off=ci * CH * 4),
                            op=mybir.AluOpType.mult)
                    else:
                        nc.vector.tensor_tensor(
                            out=APn(stg, [[P, CH], [32, 4], [1, 32]]),
                            in0=APn(pA, [[P, CH], [32, 4], [1, 32]]),
                            in1=APn(qe, [[4, CH], [1, 4], [0, 32]], off=ci * CH * 4),
                            op=mybir.AluOpType.mult)
                    cur.append(stg)

                prev = (cur, ohm)
            emit_scatter(NSC - 1, prev[0], prev[1], range(NCHK))
    nc.compile()
    return nc


def kernel(x_v, edge_index_v, x_f, edge_index_f, Wv, Uv, cv, bv, Wf, Uf, cf, bf):
    _register_ntff_hook()
    import ml_dtypes
    from concourse import bass_utils

    x_v = np.asarray(x_v, np.float32)
    x_f = np.asarray(x_f, np.float32)
    cores = []
    for bi, (x, ei, W, U, c, b) in enumerate([
            (x_v, edge_index_v, Wv, Uv, cv, bv),
            (x_f, edge_index_f, Wf, Uf, cf, bf)]):
        ei = np.asarray(ei)
        s0, d0 = ei[0].astype(np.int64), ei[1].astype(np.int64)
        m = s0 != d0
        loops = np.arange(N, dtype=np.int64)
        src = np.concatenate([s0[m], loops])
        dst = np.concatenate([d0[m], loops])
        x16 = x.astype(ml_dtypes.bfloat16)
        for j in range(4):
            lo = j * NPC
            cores.append({
                "x16": x16, "W": np.asarray(W, np.float32),
                "U": np.asarray(U, np.float32), "c": np.asarray(c, np.float32),
                "b": np.asarray(b, np.float32), "lo": lo,
                "g": _prep_core(x16, src, dst, lo),
            })

    tn = np.stack([np.ceil(c["g"]["cnt"] / P).astype(np.int64) for c in cores])
    TPB = tn.max(axis=0)
    TPB = np.maximum(TPB, 1)
    NT = int(TPB.sum())
    pad = (-NT) % SCT
    TPB[NBLK - 1] += pad
    NT += pad
    base = np.concatenate([[0], np.cumsum(TPB)])[:-1]

    in_maps = []
    for c in cores:
        arrs = _build_core_arrays(None, c, TPB, base, NT)
        in_maps.append(arrs)

    nc = _build_program(TPB, NT)
    res = bass_utils.run_bass_kernel_spmd(
        nc, in_maps, core_ids=list(range(NCORES)),
        trace=bool(int(__import__("os").environ.get("KERNEL_TRACE", "0"))))
    kernel.last_result = res
    out_v = np.concatenate([res.results[j]["out"][:NPC] for j in range(4)])
    out_f = np.concatenate([res.results[4 + j]["out"][:NPC] for j in range(4)])
    return out_v, out_f
